# revision 5
# baseline (speedup 1.0000x reference)
"""Batched quantize->matmul->dequantize kernel for 8 Trainium2 NeuronCores.

Problem: input0 [16,1024,1024] f32, input1 [16,1024,1024] f32.
  qa = clip(round(input0*10), -128, 127); qb likewise
  out = (qa @ qb) / 10            # batched, f32

Strategy: shard the batch dim across 8 cores (2 batches/core); each core runs
an identical Bass/Tile kernel with no communication.

Quantization: one multiply-by-10 with int8 output — the hardware f32->int8
conversion is round-to-nearest-even with saturation, which is exactly
jnp.clip(jnp.round(x*10), -128, 127) (verified on device incl. the
double-rounding and saturation edge cases). The int8 is then cast to bf16
for the PE: ints <= 128 are exact in bf16, products are exact in the PE's
multiply, and the fp32 PSUM accumulation of integer partial sums < 2^24 is
exact, so the matmul result matches the reference bit-for-bit (up to the
final x0.1 vs /10, <= 1 ulp).

Dequant (x0.1) is fused into the mandatory PSUM->SBUF eviction on the
scalar engine.

The A operand is laid out [b, K, M] host-side during sharding (the PE's
native stationary-operand layout: matmul computes lhsT.T @ rhs with the
contraction dim on partitions for both operands).

Engine budget per core (measured per-op costs): DVE ~48 ops (A quant both
steps + B int8->bf16), ACT ~32 ops (B mul->int8 + dequant evictions), PE 256
matmuls of [128k,128m]x[128k,512n], DMA 24 MiB. GPSIMD is left idle on
purpose: its tensor ops run ~15us/tile and its SBUF port lock stalls DVE.
"""

import sys

if "/opt/trn_rl_repo" not in sys.path:
    sys.path.insert(0, "/opt/trn_rl_repo")

import numpy as np

import concourse.bass as bass
import concourse.mybir as mybir
import concourse.tile as tile
from concourse import bacc
from concourse.bass_utils import run_bass_kernel_spmd

N_CORES = 8
B, M, K, N = 16, 1024, 1024, 1024
BPC = B // N_CORES  # batches per core
P = 128
KT = K // P  # k tiles per batch
MT = M // P  # m tiles per batch

DSCALE = 10.0
WSCALE = 10.0
OSCALE = 10.0

f32 = mybir.dt.float32
bf16 = mybir.dt.bfloat16
i8 = mybir.dt.int8


def _build_kernel(nc: bass.Bass):
    # A arrives pre-arranged [BPC, K, M]; B natural [BPC, K, N].
    a_dram = nc.dram_tensor("input0_t", [BPC, K, M], f32, kind="ExternalInput").ap()
    b_dram = nc.dram_tensor("input1", [BPC, K, N], f32, kind="ExternalInput").ap()
    c_dram = nc.dram_tensor("output", [BPC, M, N], f32, kind="ExternalOutput").ap()

    KP = KT // 2  # k-tile pairs per batch (1 MiB DMA per pair)

    with tile.TileContext(nc) as tc:
        with (
            tc.tile_pool(name="warm", bufs=1) as warm_pool,
            tc.tile_pool(name="wpsum", bufs=1, space="PSUM") as wpsum_pool,
            tc.tile_pool(name="a_f32", bufs=4) as a_pool,
            tc.tile_pool(name="b_f32", bufs=4) as b_pool,
            tc.tile_pool(name="a_i8", bufs=3) as ai_pool,
            tc.tile_pool(name="b_i8", bufs=3) as bi_pool,
            tc.tile_pool(name="qa", bufs=BPC * KP) as qa_pool,
            tc.tile_pool(name="qb", bufs=BPC * KP) as qb_pool,
            tc.tile_pool(name="psum", bufs=3, space="PSUM") as psum_pool,
            tc.tile_pool(name="c_f32", bufs=6) as c_pool,
        ):
            # PE warmup: keep the PE busy from t~0 so the HAM clock gate is
            # released (2.4 GHz) by the time real matmuls are ready, instead
            # of paying ~2x cadence on the first ~3.4us of real work.
            wsrc = warm_pool.tile([P, 512], bf16)
            nc.vector.memset(wsrc[:], 0.0)
            wps = wpsum_pool.tile([P, 512], f32)
            for _ in range(48):
                nc.tensor.matmul(wps[:], wsrc[:, :P], wsrc[:], start=True, stop=True)

            for b in range(BPC):
                qa = []
                qb = []
                for kp in range(KP):
                    # one 1 MiB DMA loads two k-tiles: SBUF [128, 2048] where
                    # free half t holds HBM rows (2*kp+t)*128 + p
                    a_src = a_dram[b, 2 * kp * P : (2 * kp + 2) * P, :].rearrange(
                        "(two p) n -> p two n", p=P
                    )
                    at = a_pool.tile([P, 2 * M], f32)
                    nc.sync.dma_start(
                        out=at[:].rearrange("p (two n) -> p two n", two=2),
                        in_=a_src,
                    )
                    ai = ai_pool.tile([P, 2 * M], i8)
                    # f32->int8 convert = RNE + saturate == clip(round(10x))
                    nc.vector.tensor_scalar_mul(ai[:], at[:], DSCALE)
                    qat = qa_pool.tile([P, 2 * M], bf16)
                    nc.vector.tensor_copy(out=qat[:], in_=ai[:])
                    qa.append(qat)

                    b_src = b_dram[b, 2 * kp * P : (2 * kp + 2) * P, :].rearrange(
                        "(two p) n -> p two n", p=P
                    )
                    bt = b_pool.tile([P, 2 * N], f32)
                    nc.sync.dma_start(
                        out=bt[:].rearrange("p (two n) -> p two n", two=2),
                        in_=b_src,
                    )
                    bi = bi_pool.tile([P, 2 * N], i8)
                    nc.scalar.activation(
                        bi[:],
                        bt[:],
                        mybir.ActivationFunctionType.Copy,
                        scale=WSCALE,
                    )
                    qbt = qb_pool.tile([P, 2 * N], bf16)
                    nc.vector.tensor_copy(out=qbt[:], in_=bi[:])
                    qb.append(qbt)

                for m in range(MT):
                    ps = psum_pool.tile([P, N], f32)
                    for k in range(KT):
                        kp, t = divmod(k, 2)
                        lhsT = qa[kp][:, t * M + m * P : t * M + (m + 1) * P]
                        for nh in range(2):
                            nc.tensor.matmul(
                                ps[:, nh * 512 : (nh + 1) * 512],
                                lhsT,
                                qb[kp][:, t * N + nh * 512 : t * N + (nh + 1) * 512],
                                start=(k == 0),
                                stop=(k == KT - 1),
                            )
                    ct = c_pool.tile([P, N], f32)
                    # dequant fused into the mandatory PSUM->SBUF eviction
                    nc.scalar.activation(
                        ct[:],
                        ps[:],
                        mybir.ActivationFunctionType.Copy,
                        scale=1.0 / OSCALE,
                    )
                    nc.sync.dma_start(
                        out=c_dram[b, m * P : (m + 1) * P, :], in_=ct[:]
                    )


_NC_CACHE = None


def _get_nc():
    global _NC_CACHE
    if _NC_CACHE is None:
        nc = bacc.Bacc("TRN2", target_bir_lowering=False, debug=False,
                       num_devices=N_CORES)
        _build_kernel(nc)
        nc.compile()
        _NC_CACHE = nc
    return _NC_CACHE


def _make_in_maps(input0: np.ndarray, input1: np.ndarray):
    in_maps = []
    for c in range(N_CORES):
        sl = slice(c * BPC, (c + 1) * BPC)
        a_t = np.ascontiguousarray(input0[sl].transpose(0, 2, 1))
        in_maps.append(
            {"input0_t": a_t, "input1": np.ascontiguousarray(input1[sl])}
        )
    return in_maps


def kernel(input0, input1, **run_kwargs):
    input0 = np.asarray(input0, dtype=np.float32)
    input1 = np.asarray(input1, dtype=np.float32)
    assert input0.shape == (B, M, K) and input1.shape == (B, K, N)

    nc = _get_nc()
    res = run_bass_kernel_spmd(
        nc, _make_in_maps(input0, input1), core_ids=list(range(N_CORES)),
        **run_kwargs,
    )
    out = np.concatenate(
        [res.results[c]["output"] for c in range(N_CORES)], axis=0
    )
    if run_kwargs:
        return out, res
    return out


if __name__ == "__main__":
    a = np.random.randn(B, M, K).astype(np.float32)
    bm = np.random.randn(B, K, N).astype(np.float32)
    out = kernel(a, bm)
    print("out", out.shape, out.dtype)
